# revision 20
# baseline (speedup 1.0000x reference)
"""Trainium2 Bass kernel for nn_BiMambaBlock (8-core data-parallel over batch).

Model: 2 BiMamba layers + transition MLP block + RMSNorm, x [8, 3072, 256].
Batch dim == 8 == n_cores -> one batch element per core, no collectives.

Per-core layout is channel-major [c, 3072], the 3 folded segments seg-major
along the free dim.  Host-side weight algebra:
  - depthwise conv1d folded into in_proj: 6 tap-matmuls with row-scaled
    weights (conv commutes with the preceding dense projection), silu applied
    during PSUM extraction
  - depthwise 3x3 conv folded into fc1 the same way (9 tap-matmuls, bf16),
    gelu applied during PSUM extraction
  - dt_proj @ x_proj[dt-rows] folded into one W_delta matmul
  - D-skip and z-gate folded into out_proj weights
  - x_proj B/C rows replicated to (n, d_lo) matmul weights; full 128-way
    replication via a DRAM-bounce broadcast DMA
  - y = sum_n C_n (hf_n + hb_n) folded into out_proj as 16 extra K-chunks
dA = exp(A_n * delta) via ACT Exp with per-partition scale.  The recurrence
runs on the native tensor_tensor_scan (DVE forward, GPSIMD backward, the
backward direction through negative-step access patterns).
Segment-edge zero-padding of the convs is realized with N=511 edge matmuls
and PSUM has_written semantics (full-coverage center tap issued first).
"""

import numpy as np

import concourse.bass as bass
import concourse.bacc as bacc
import concourse.mybir as mybir
import concourse.tile as tile
from concourse.bass_utils import run_bass_kernel_spmd

import ml_dtypes

BF16 = np.dtype(ml_dtypes.bfloat16)

F32 = mybir.dt.float32
F32R = mybir.dt.float32r
BF = mybir.dt.bfloat16
AF = mybir.ActivationFunctionType
OP = mybir.AluOpType

P = 128          # partitions / d_inner
DIM = 256
NST = 16         # d_state
T = 1024         # tokens per folded segment
NSEG = 3
L = NSEG * T
TPD = T + 2     # padded segment
NCORES = 8
EPS = 1e-5

# (out_col_offset, rhs_col_offset, N) per (tau-shift k, half) for the
# edge-aware 512-column matmul chunks; k=1 is the full-coverage center.
def _tau_chunk(k, half):
    if k == 0:
        return (1, 0, 511) if half == 0 else (0, 511, 512)
    if k == 1:
        return (0, half * 512, 512)
    return (0, 1, 512) if half == 0 else (0, 513, 511)


class _WPack:
    def __init__(self):
        self.cols = []
        self.off = {}
        self.n = 0

    def add(self, name, arr):
        arr = np.ascontiguousarray(arr, dtype=np.float32)
        assert arr.shape[0] == P, (name, arr.shape)
        self.off[name] = self.n
        self.n += arr.shape[1]
        self.cols.append(arr)

    def pack(self):
        return np.concatenate(self.cols, axis=1)


def _host_prep(layers, mlp, norm_w):
    wp = _WPack()
    for li, p in enumerate(layers):
        p = {k: np.asarray(v, dtype=np.float32) for k, v in p.items()}
        # conv1d folded into in_proj: tap (j, k) weight[g, i] =
        #   conv_w[g, j, k] * in_proj_w[2g+j, i]
        assert np.abs(p["in_proj_b"]).max() == 0.0
        g = np.arange(P)
        for j in range(2):
            rows = p["in_proj_w"][2 * g + j]            # [128, 256]
            for k in range(3):
                w1d = rows * p["conv_w"][:, j, k][:, None]
                wp.add(f"w1d{j}{k}k0_{li}", w1d.T[:P])   # lhsT chunks
                wp.add(f"w1d{j}{k}k1_{li}", w1d.T[P:])
        wp.add(f"ub_{li}", p["conv_b"][:, None])         # silu bias
        wp.add(f"acol_{li}", -np.exp(p["A_log"]))
        wp.add(f"wd_{li}", (p["dt_proj_w"] @ p["x_proj_w"][:NST]).T)
        wp.add(f"dtb_{li}", p["dt_proj_b"][:, None])
        m = np.arange(P)
        wp.add(f"wb8_{li}", p["x_proj_w"][NST + m // 8].T)
        wp.add(f"wc8_{li}", p["x_proj_w"][2 * NST + m // 8].T)
        woy = p["out_proj_w"][:, :P]
        woz = p["out_proj_w"][:, P:] + woy * (2.0 * p["D"])[None, :]
        wp.add(f"woz_{li}", woz.T)
        wp.add(f"opb_{li}", p["out_proj_b"].reshape(2, P).T)
    m = {k: np.asarray(v, dtype=np.float32) for k, v in mlp.items()}
    assert np.abs(m["fc1_b"]).max() == 0.0
    wp.add("c2b", m["conv_b"].reshape(8, P).T)
    wp.add("f2b", m["fc2_b"].reshape(2, P).T)
    wp.add("normw", np.asarray(norm_w, np.float32).reshape(2, P).T)
    wp.add("ones", np.ones((P, P), np.float32))
    wp.add("zero", np.zeros((P, 1), np.float32))
    wp.add("epsc", np.full((P, 1), EPS, np.float32))
    wk = wp.pack()

    # bf16 block: fc2_w.T (8 K-chunks) + out_proj-y lhsT per layer
    wf2t = m["fc2_w"].T
    blocks = [wf2t[kc * P:(kc + 1) * P] for kc in range(8)]
    for p in layers:
        blocks.append(np.asarray(p["out_proj_w"], np.float32)[:, :P].T)
    wf2 = np.concatenate(blocks, axis=1).astype(BF16)

    # 3x3 conv folded into fc1: tap (df, dt) weight[o, i] =
    #   conv_w[o, 0, df, dt] * fc1_w[o, i];  lhsT [256, 1024] per tap, bf16
    w2 = m["conv_w"].reshape(1024, 9)
    f1 = m["fc1_w"]                                      # [1024, 256]
    taps = []
    for t in range(9):
        wt = (f1 * w2[:, t][:, None]).T                  # [256, 1024]
        taps.append(wt[:P])
        taps.append(wt[P:])
    wt9 = np.concatenate(taps, axis=1).astype(BF16)      # [128, 18*1024]
    return wk, wp.off, wf2, wt9


EXPC9 = 9 * 1024


def _build(off, wkcols):
    nc = bacc.Bacc(None, target_bir_lowering=False, debug=False)
    x_d = nc.dram_tensor("xin", [DIM, L], F32R, kind="ExternalInput")
    wk_d = nc.dram_tensor("wk", [P, wkcols], F32R, kind="ExternalInput")
    wf2_d = nc.dram_tensor("wf2", [P, 10 * DIM], BF, kind="ExternalInput")
    wt9_d = nc.dram_tensor("wt9", [P, 18 * 1024], BF, kind="ExternalInput")
    out_d = nc.dram_tensor("out", [DIM, L], F32, kind="ExternalOutput")
    bounce = [(nc.dram_tensor(f"b8d{li}", [P, L], BF),
               nc.dram_tensor(f"c8d{li}", [P, L], BF)) for li in range(2)]

    with tile.TileContext(nc) as tc:
        with (
            tc.tile_pool(name="persist", bufs=1) as pers,
            tc.tile_pool(name="psum", bufs=6, space="PSUM") as psum,
        ):
            wk = pers.tile([P, wkcols], F32R)
            nc.sync.dma_start(wk[:], wk_d[:])
            wf2 = pers.tile([P, 10 * DIM], BF)
            nc.sync.dma_start(wf2[:], wf2_d[:])
            wob = wf2[:, 8 * DIM:]
            x0 = pers.tile([P, L], F32R)
            x1 = pers.tile([P, L], F32R)
            nc.sync.dma_start(x0[:], x_d[:P, :])
            nc.sync.dma_start(x1[:], x_d[P:, :])
            xts = (x0, x1)

            def wkc(name, n=1, li=None):
                key = name if li is None else f"{name}_{li}"
                o = off[key]
                return wk[:, o:o + n]

            def wkf(name, n=1, li=None):
                return wkc(name, n, li).bitcast(F32)

            # ---------------- BiMamba layers ----------------
            for li in range(2):
                b8d, c8d = bounce[li]
                with tc.tile_pool(name=f"lay{li}", bufs=1) as lay:
                    # fused in_proj + conv1d + silu -> u  (u also = z)
                    # x copied into a zero-padded per-segment layout so every
                    # tap is a full N=512 matmul reading pad zeros at edges.
                    xp0 = lay.tile([P, NSEG * TPD], F32R, name="xp0")
                    xp1 = lay.tile([P, NSEG * TPD], F32R, name="xp1")
                    for xp, xt in ((xp0, x0), (xp1, x1)):
                        nc.gpsimd.memset(xp[:].bitcast(F32), 0.0)
                        v = xp[:].rearrange("p (s c) -> p s c", s=NSEG)
                        nc.vector.tensor_copy(
                            v[:, :, 1:1 + T],
                            xt[:].rearrange("p (s t) -> p s t", s=NSEG))
                    xps = (xp0, xp1)
                    u = lay.tile([P, L], F32R)
                    for s in range(NSEG):
                        for half in range(2):
                            pt = psum.tile([P, 512], F32)
                            mms = []
                            for k in (1, 0, 2):
                                for j in range(2):
                                    for kc in range(2):
                                        lhsT = wkc(f"w1d{j}{k}k{kc}", P, li)
                                        ro = s * TPD + k + half * 512
                                        rhs = xps[kc][:, ro: ro + 512]
                                        mms.append((pt[:], lhsT, rhs))
                            for i, (o_, l_, r_) in enumerate(mms):
                                nc.tensor.matmul(o_, l_, r_, start=(i == 0),
                                                 stop=(i == len(mms) - 1))
                            nc.scalar.activation(
                                u[:, s * T + half * 512: s * T + half * 512 + 512],
                                pt[:], AF.Silu, bias=wkf("ub", 1, li))

                    # delta = softplus(W_delta @ u + dtb) = ln(1 + exp(.))
                    delta = lay.tile([P, L], F32)
                    wd = wkc("wd", P, li)
                    for nb in range(6):
                        sl = slice(nb * 512, (nb + 1) * 512)
                        pt = psum.tile([P, 512], F32)
                        nc.tensor.matmul(pt[:], wd, u[:, sl], start=True, stop=True)
                        nc.scalar.activation(delta[:, sl], pt[:], AF.Exp,
                                             bias=wkf("dtb", 1, li))
                        nc.scalar.activation(delta[:, sl], delta[:, sl], AF.Ln,
                                             bias=wkf("ones")[:, :1])
                    du = lay.tile([P, L], BF)
                    nc.vector.tensor_tensor(du[:], delta[:], u[:], OP.mult)

                    # B8/C8 [(n, d_lo), tau] -> DRAM bounce (bf16)
                    for wname, dram in (("wb8", b8d), ("wc8", c8d)):
                        w8 = wkc(wname, P, li)
                        t8 = lay.tile([P, L], BF, tag="t8")
                        for nb in range(6):
                            pt = psum.tile([P, 512], F32)
                            nc.tensor.matmul(pt[:], w8, u[:, nb * 512:(nb + 1) * 512],
                                             start=True, stop=True)
                            nc.scalar.activation(t8[:, nb * 512:(nb + 1) * 512], pt[:],
                                                 AF.Identity, bias=wkf("zero"))
                        nc.sync.dma_start(dram[:], t8[:])

                    # selective scan fwd+bwd; y-contraction folded into out_proj
                    acol = wkf("acol", NST, li)
                    woz = wkc("woz", DIM, li)
                    opb = wkf("opb", 2, li)
                    with (
                        tc.tile_pool(name=f"scan{li}", bufs=3) as sp,
                        tc.tile_pool(name=f"scanh{li}", bufs=2) as hp,
                        tc.tile_pool(name=f"hc{li}", bufs=1) as hcp,
                    ):
                        for s in range(NSEG):
                            sl = slice(s * T, (s + 1) * T)
                            hcs = []
                            for n in range(NST):
                                brep = sp.tile([P, T], BF, tag="brep")
                                nc.sync.dma_start(brep[:], bass.AP(
                                    tensor=b8d, offset=8 * n * L + s * T,
                                    ap=[[0, NST], [L, 8], [1, T]]))
                                crep = sp.tile([P, T], BF, tag="crep")
                                nc.sync.dma_start(crep[:], bass.AP(
                                    tensor=c8d, offset=8 * n * L + s * T,
                                    ap=[[0, NST], [L, 8], [1, T]]))
                                dA = sp.tile([P, T], F32, tag="dA")
                                nc.scalar.activation(dA[:], delta[:, sl], AF.Exp,
                                                     bias=wkf("zero"),
                                                     scale=acol[:, n:n + 1])
                                dBu = sp.tile([P, T], BF, tag="dBu")
                                nc.vector.tensor_tensor(dBu[:], du[:, sl], brep[:],
                                                        OP.mult)
                                hf = hp.tile([P, T], BF, tag="hf")
                                hb = hp.tile([P, T], BF, tag="hb")
                                fwd_eng = bwd_eng = nc.vector
                                fwd_eng.tensor_tensor_scan(
                                    hf[:], dA[:], dBu[:], 0.0, OP.mult, OP.add)
                                bwd_eng.tensor_tensor_scan(
                                    hb[:][:, ::-1], dA[:][:, ::-1],
                                    dBu[:][:, ::-1], 0.0, OP.mult, OP.add)
                                hc = hcp.tile([P, T], BF, tag=f"hc{n}")
                                nc.vector.tensor_tensor(hf[:], hf[:], hb[:], OP.add)
                                nc.vector.tensor_tensor(hc[:], hf[:], crep[:], OP.mult)
                                hcs.append(hc)
                            for mc in range(2):
                                for half in range(2):
                                    hsl = slice(half * 512, (half + 1) * 512)
                                    xsl = slice(s * T + half * 512,
                                                s * T + half * 512 + 512)
                                    pt = psum.tile([P, 512], F32)
                                    for n in range(NST):
                                        nc.tensor.matmul(
                                            pt[:],
                                            wob[:, li * DIM + mc * P:
                                                li * DIM + (mc + 1) * P],
                                            hcs[n][:, hsl],
                                            start=(n == 0), stop=False)
                                    nc.tensor.matmul(
                                        pt[:], woz[:, mc * P:(mc + 1) * P],
                                        u[:, xsl], start=False, stop=True)
                                    nc.vector.scalar_tensor_tensor(
                                        xts[mc][:, xsl], pt[:], opb[:, mc:mc + 1],
                                        xts[mc][:, xsl], OP.add, OP.add)

            # ---------------- RMSNorm + transition ----------------
            ones = wkc("ones", P)
            normw = wkf("normw", 2)

            def rmsnorm(dsts, rp, dst_sl=None):
                if dst_sl is None:
                    dst_sl = lambda nb: slice(nb * 512, (nb + 1) * 512)
                sq = rp.tile([P, L], F32R, tag="rnsq", name="rnsq")
                rnrow = rp.tile([1, L], F32R, tag="rnrow", name="rnrow")
                rsp = rp.tile([P, L // P], F32R, tag="rnsp", name="rnsp")
                for nb in range(6):
                    sl = slice(nb * 512, (nb + 1) * 512)
                    pt = psum.tile([P, 512], F32)
                    nc.scalar.activation(sq[:, sl], x0[:, sl], AF.Square,
                                         bias=wkf("zero"))
                    nc.tensor.matmul(pt[:1, :], ones[:, :1], sq[:, sl],
                                     start=True, stop=False)
                    nc.scalar.activation(sq[:, sl], x1[:, sl], AF.Square,
                                         bias=wkf("zero"))
                    nc.tensor.matmul(pt[:1, :], ones[:, :1], sq[:, sl],
                                     start=False, stop=True)
                    nc.scalar.activation(rnrow[:, sl], pt[:1, :], AF.Sqrt,
                                         bias=wkf("epsc")[:1, :], scale=1.0 / DIM)
                nc.sync.dma_start(rsp[:], rnrow[:1, :])
                nc.vector.reciprocal(rsp[:].bitcast(F32), rsp[:].bitcast(F32))
                nc.sync.dma_start(rnrow[:1, :], rsp[:])
                for nb in range(6):
                    sl = slice(nb * 512, (nb + 1) * 512)
                    pt = psum.tile([P, 512], F32)
                    nc.tensor.matmul(pt[:], ones[:1, :], rnrow[:, sl],
                                     start=True, stop=True)
                    for mc in range(2):
                        nc.vector.scalar_tensor_tensor(
                            dsts[mc][:, dst_sl(nb)], xts[mc][:, sl],
                            normw[:, mc:mc + 1], pt[:], OP.mult, OP.mult)

            with tc.tile_pool(name="trans", bufs=1) as tp:
                xb0 = tp.tile([P, NSEG * TPD], BF)
                xb1 = tp.tile([P, NSEG * TPD], BF)
                nc.gpsimd.memset(xb0[:], 0.0)
                nc.gpsimd.memset(xb1[:], 0.0)
                wt9 = tp.tile([P, 18 * 1024], BF)
                nc.sync.dma_start(wt9[:], wt9_d[:])
                with tc.tile_pool(name="rn1", bufs=1) as rp:
                    rmsnorm((xb0, xb1), rp,
                            dst_sl=lambda nb: slice(
                                (nb // 2) * TPD + 1 + (nb % 2) * 512,
                                (nb // 2) * TPD + 1 + (nb % 2) * 512 + 512))
                xbs = (xb0, xb1)

                # fused fc1 + 3x3 conv + gelu -> g (bf16)
                c2b = wkf("c2b", 8)
                with tc.tile_pool(name="gpool", bufs=1) as gp:
                    gs = []
                    for mc in range(8):
                        g = gp.tile([P, L], BF, tag=f"g{mc}", name=f"g{mc}")
                        for s in range(NSEG):
                            for half in range(2):
                                pt = psum.tile([P, 512], F32)
                                mms = []
                                for df in (1, 0, 2):
                                    sf = s + df - 1
                                    if not 0 <= sf < NSEG:
                                        continue
                                    for dt_ in (1, 0, 2):
                                        t = df * 3 + dt_
                                        ro = sf * TPD + dt_ + half * 512
                                        for kc in range(2):
                                            lhsT = wt9[:, (t * 2 + kc) * T + mc * P:
                                                       (t * 2 + kc) * T + (mc + 1) * P]
                                            rhs = xbs[kc][:, ro: ro + 512]
                                            mms.append((pt[:], lhsT, rhs))
                                for i, (o_, l_, r_) in enumerate(mms):
                                    nc.tensor.matmul(o_, l_, r_, start=(i == 0),
                                                     stop=(i == len(mms) - 1))
                                nc.scalar.activation(
                                    g[:, s * T + half * 512: s * T + half * 512 + 512],
                                    pt[:], AF.Gelu, bias=c2b[:, mc:mc + 1])
                        gs.append(g)

                    # fc2 + residual into x
                    f2b = wkf("f2b", 2)
                    for mc in range(2):
                        for nb in range(6):
                            sl = slice(nb * 512, (nb + 1) * 512)
                            pt = psum.tile([P, 512], F32)
                            for kc in range(8):
                                nc.tensor.matmul(
                                    pt[:],
                                    wf2[:, kc * DIM + mc * P: kc * DIM + (mc + 1) * P],
                                    gs[kc][:, sl], start=(kc == 0), stop=(kc == 7))
                            nc.vector.scalar_tensor_tensor(
                                xts[mc][:, sl], pt[:], f2b[:, mc:mc + 1],
                                xts[mc][:, sl], OP.add, OP.add)

                # final rmsnorm -> output
                on0 = tp.tile([P, L], F32)
                on1 = tp.tile([P, L], F32)
                with tc.tile_pool(name="rn2", bufs=1) as rp:
                    rmsnorm((on0, on1), rp)
                nc.sync.dma_start(out_d[:P, :], on0[:])
                nc.sync.dma_start(out_d[P:, :], on1[:])

    nc.compile()
    return nc


_CACHE = {}


def _get_nc(wkcols, off):
    if wkcols not in _CACHE:
        _CACHE[wkcols] = _build(off, wkcols)
    return _CACHE[wkcols]


def _run(x, layers, mlp, norm_w, trace=False):
    x = np.asarray(x, dtype=np.float32)
    wk, off, wf2, wt9 = _host_prep(layers, mlp, norm_w)
    nc = _get_nc(wk.shape[1], off)
    in_maps = []
    for b in range(NCORES):
        in_maps.append({
            "xin": np.ascontiguousarray(x[b].T),
            "wk": wk,
            "wf2": np.ascontiguousarray(wf2),
            "wt9": np.ascontiguousarray(wt9),
        })
    return run_bass_kernel_spmd(nc, in_maps, list(range(NCORES)), trace=trace)


def kernel(x, layers, mlp, norm_w):
    res = _run(x, layers, mlp, norm_w)
    return np.stack([np.asarray(res.results[b]["out"], dtype=np.float32).T
                     for b in range(NCORES)])


def profile_once(x, layers, mlp, norm_w):
    try:
        res = _run(x, layers, mlp, norm_w, trace=True)
        return res.exec_time_ns
    except Exception:
        return None
